# revision 24
# baseline (speedup 1.0000x reference)
"""CaptioningRNN (attention LSTM + vocab softmax loss) on 8 TRN2 NeuronCores.

Data-parallel over batch N=256 -> 32 samples/core. Weights replicated.
All matmuls bf16 (fp32 PSUM accumulate). Per-core partial losses summed on host.

Layouts (per core, B=32 samples, S=31 steps, H=1024, P16=16 spatial):
  - GEMM orientation: out = lhsT.T @ rhs with lhsT = inputT slices
    (feature-dim on partitions, batch on lhsT free), rhs = weight slices
    (feature-dim on partitions, out-cols free). Gate psum tiles (128,512)
    pack 4 units of 32 batch rows via PE col-tiling (tile_position).
  - Attention uses an all-pairs score matmul masked by a constant M32
    (block-diag mask added via an identity matmul), softmax on ACT with
    fused accumulate, and a precomputed B = A2 @ Wattn so the per-step
    attention context enters the gate GEMM as 4 extra K-chunks.
"""

import os
import numpy as np
import ml_dtypes

BF = ml_dtypes.bfloat16
F8 = ml_dtypes.float8_e4m3  # IEEE e4m3 max±240 — matches TRN FP8_EXP4

N, T, V, W_DIM, H, D_IMG = 256, 32, 10000, 512, 1024, 1280
P16 = 16
NC = 8
B = N // NC          # 32 samples per core
S = T - 1            # 31 steps
ROWS = B * S         # 992 (t,n) rows per core, r = 32*t + n
MCH = 8              # vocab row chunks
MROW = ROWS // MCH   # 124
VCH = 20             # vocab col chunks
VCOL = V // VCH      # 500
NEG = -1.0e5         # mask value (exp underflows to exactly 0)

_cache = {}

last_exec_ns = None


def _build(has_b, has_bvocab, phases=3):
    import concourse.mybir as mybir
    from concourse.bacc import Bacc
    from concourse.tile import TileContext

    F32 = mybir.dt.float32
    BF16 = mybir.dt.bfloat16
    FP8 = mybir.dt.float8e4
    DR = mybir.MatmulPerfMode.DoubleRow
    AF = mybir.ActivationFunctionType
    ALU = mybir.AluOpType
    AX = mybir.AxisListType

    nc = Bacc()

    # ---- dram parameters (per-core shapes) ----
    d_f2t = nc.declare_dram_parameter("f2t", [1536, 512], FP8, isOutput=False)
    d_wproj = nc.declare_dram_parameter("wproj", [1536, 1024], FP8, isOutput=False)
    d_wattn = nc.declare_dram_parameter("wattn", [1024, 4096], FP8, isOutput=False)
    d_wh = nc.declare_dram_parameter("wh", [1024, 4096], BF16, isOutput=False)
    d_wx = nc.declare_dram_parameter("wx", [512, 4096], BF16, isOutput=False)
    d_xt = nc.declare_dram_parameter("xt", [512, ROWS], BF16, isOutput=False)
    d_wvoc = nc.declare_dram_parameter("wvoc", [1024, V], FP8, isOutput=False)
    d_wtgt = nc.declare_dram_parameter("wtgt", [1024, ROWS], BF16, isOutput=False)
    d_maskm = nc.declare_dram_parameter("maskm", [MROW, MCH], F32, isOutput=False)
    d_i128 = nc.declare_dram_parameter("i128", [128, 128], BF16, isOutput=False)
    d_m32 = nc.declare_dram_parameter("m32", [32, 512], BF16, isOutput=False)
    if has_b:
        d_bvec = nc.declare_dram_parameter("bvec", [1, 4096], BF16, isOutput=False)
    if has_bvocab:
        d_bvoc = nc.declare_dram_parameter("bvoc", [1, V], BF16, isOutput=False)
        d_btgt = nc.declare_dram_parameter("btgt", [1, ROWS], F32, isOutput=False)
    d_loss = nc.declare_dram_parameter("loss", [1, 1], F32, isOutput=True)

    with TileContext(nc) as tc:
        with tc.tile_pool(name="ppa", bufs=1) as ppa:
            # ---- persistent tiles (live across phases) ----
            at8_t = ppa.tile([128, 8, 512], FP8, tag="at8")       # A2T (H-chunks, (n,p))
            bp_t = [ppa.tile([128, 4096], BF16, tag=f"bp{c}", name=f"bp{c}") for c in range(4)]
            hst_t = ppa.tile([128, 8, ROWS], BF16, tag="hst")      # hsT history
            hst8_t = ppa.tile([128, 8, ROWS], FP8, tag="hst8")
            h0t_t = ppa.tile([128, 8, B], BF16, tag="h0t")
            h0t8_t = ppa.tile([128, 8, B], FP8, tag="h0t8")
            c_t = ppa.tile([64, 512], F32, tag="cst")              # LSTM c state
            i128_t = ppa.tile([128, 128], BF16, tag="i128")
            m32_t = ppa.tile([32, 512], BF16, tag="m32")
            ones_t = ppa.tile([1, 128], BF16, tag="ones")
            nc.sync.dma_start(i128_t[:], d_i128[:])
            nc.sync.dma_start(m32_t[:], d_m32[:])
            nc.vector.memset(ones_t[:], 1.0)
            if has_b:
                bvec_t = ppa.tile([1, 4096], BF16, tag="bvec")
                nc.sync.dma_start(bvec_t[:], d_bvec[:])

            # ================= P1: feature projection -> A2T, h0, c0 ==========
            with (
                tc.tile_pool(name="p12", bufs=1) as p12,
                tc.tile_pool(name="psa", bufs=2, space="PSUM") as psa,
            ):
                wproj_t = p12.tile([128, 12, 1024], FP8, tag="wproj")
                f2t_t = p12.tile([128, 12, 512], FP8, tag="f2t")
                h0f_t = p12.tile([128, 8, B], F32, tag="h0f")
                nc.sync.dma_start(
                    wproj_t[:], d_wproj[:].rearrange("(c k) m -> k c m", k=128))
                nc.sync.dma_start(
                    f2t_t[:], d_f2t[:].rearrange("(c k) m -> k c m", k=128))
                for hc in range(8):
                    ps = psa.tile([128, 512], F32, tag="pp")
                    for kk in range(6):
                        nc.tensor.matmul(
                            ps[:],
                            wproj_t[:, 2 * kk:2 * kk + 2,
                                    128 * hc:128 * (hc + 1)],
                            f2t_t[:, 2 * kk:2 * kk + 2, :],
                            start=(kk == 0), stop=(kk == 5), perf_mode=DR)
                    nc.scalar.copy(at8_t[:, hc, :], ps[:])
                    nc.vector.reduce_sum(
                        h0f_t[:, hc, :],
                        ps[:].rearrange("k (n p) -> k n p", p=P16),
                        axis=AX.X)
                nc.vector.tensor_scalar(h0t_t[:], h0f_t[:],
                                        1.0 / P16, None, op0=ALU.mult)
                nc.gpsimd.tensor_scalar(h0t8_t[:], h0f_t[:],
                                        1.0 / P16, None, op0=ALU.mult)
                c0p = psa.tile([64, 512], BF16, tag="c0p")
                for k in range(8):
                    eta, j = k // 4, k % 4
                    nc.tensor.transpose(
                        c0p[32 * eta:32 * (eta + 1), 128 * j:128 * (j + 1)],
                        h0t_t[:, k, :], i128_t[:, 0:128],
                        tile_position=(0, 32 * eta))
                nc.vector.tensor_copy(c_t[:], c0p[:])

                # ================= P2: B = A2 @ Wattn ==========
                with tc.tile_pool(name="p2w", bufs=2) as p2w:
                    for nch in range(8):
                        wat_t = p2w.tile([128, 8, 512], FP8, tag="wat")
                        nc.sync.dma_start(
                            wat_t[:],
                            d_wattn[:, 512 * nch:512 * (nch + 1)]
                            .rearrange("(c k) m -> k c m", k=128))
                        for c in range(4):
                            ps = psa.tile([128, 512], F32, tag="pp")
                            for k in range(4):
                                nc.tensor.matmul(
                                    ps[:],
                                    at8_t[:, 2 * k:2 * k + 2,
                                          128 * c:128 * (c + 1)],
                                    wat_t[:, 2 * k:2 * k + 2, :],
                                    start=(k == 0), stop=(k == 3),
                                    perf_mode=DR)
                            nc.vector.tensor_copy(
                                bp_t[c][:, 512 * nch:512 * (nch + 1)], ps[:])

            # ================= P3: recurrence ==========
            if phases < 2:
                dbg_t = ppa.tile([1, 1], F32, tag="dbg")
                nc.vector.tensor_copy(dbg_t[:], c_t[0:1, 0:1])
                nc.sync.dma_start(d_loss[:], dbg_t[:])
            if phases >= 2:
              with (
                  tc.tile_pool(name="ppb", bufs=1) as ppb,
                  tc.tile_pool(name="ps3", bufs=2, space="PSUM") as ps3,
                  tc.tile_pool(name="ps3s", bufs=2, space="PSUM") as ps3s,
                  tc.tile_pool(name="wk3", bufs=3) as wk3,
                  tc.tile_pool(name="wk3g", bufs=1) as wk3g,
                  tc.tile_pool(name="wk3h", bufs=1) as wk3h,
              ):
                  wh_t = ppb.tile([128, 8, 4096], BF16, tag="wh")
                  wx_t = ppb.tile([128, 4, 4096], BF16, tag="wx")
                  xt_t = ppb.tile([128, 4, ROWS], BF16, tag="xt")
                  nc.sync.dma_start(
                      wh_t[:], d_wh[:].rearrange("(c k) m -> k c m", k=128))
                  nc.sync.dma_start(
                      wx_t[:], d_wx[:].rearrange("(c k) m -> k c m", k=128))
                  nc.sync.dma_start(
                      xt_t[:], d_xt[:].rearrange("(c k) m -> k c m", k=128))

                  def ht_lhs(t, k):
                      if t == 0:
                          return h0t_t[:, k, :]
                      return hst_t[:, k, B * (t - 1):B * t]

                  def ht8_pair(t, c):
                      if t == 0:
                          return h0t8_t[:, 2 * c:2 * c + 2, :]
                      return hst8_t[:, 2 * c:2 * c + 2, B * (t - 1):B * t]

                  def emit_x(t2):
                      pA = ps3.tile([128, 512], F32, tag="pA", name=f"pA{t2}")
                      pB = ps3.tile([128, 512], F32, tag="pB", name=f"pB{t2}")
                      units2 = [(0, 0), (0, 1), (1, 0), (1, 1),
                                (2, 0), (2, 1), (3, 0), (3, 1)]
                      for c2 in range(4):
                          for u2, (g2, e2) in enumerate(units2):
                              ps2, j2 = (pA, u2) if u2 < 4 else (pB, u2 - 4)
                              lo2 = 1024 * g2 + 512 * e2
                              sl2 = slice(32 * j2, 32 * (j2 + 1))
                              nc.tensor.matmul(
                                  ps2[sl2, :], xt_t[:, c2, B * t2:B * (t2 + 1)],
                                  wx_t[:, c2, lo2:lo2 + 512],
                                  start=(c2 == 0), stop=False,
                                  tile_position=(0, 32 * j2),
                                  skip_group_check=True)
                      return pA, pB

                  ps_cur = emit_x(0)
                  for t in range(S):
                      # ---- attention scores + softmax (uses h from step t-1)
                      psS = ps3s.tile([32, 512], F32, tag="pS")
                      nc.tensor.matmul(psS[:], i128_t[0:32, 0:32], m32_t[:],
                                       start=True, stop=False)
                      for c in range(4):
                          nc.tensor.matmul(psS[:], ht8_pair(t, c),
                                           at8_t[:, 2 * c:2 * c + 2, :],
                                           start=False, stop=(c == 3),
                                           perf_mode=DR)
                      e_t = wk3.tile([32, 512], F32, tag="e")
                      se_t = wk3.tile([32, 1], F32, tag="se")
                      nc.scalar.activation(e_t[:], psS[:], AF.Exp,
                                           scale=float(1.0 / np.sqrt(H)),
                                           accum_out=se_t[:, 0:1])
                      re_t = wk3.tile([32, 1], F32, tag="re")
                      nc.vector.reciprocal(re_t[:], se_t[:])
                      w_t = wk3.tile([32, 512], BF16, tag="w")
                      nc.vector.tensor_scalar(w_t[:], e_t[:], re_t[:, 0:1], None,
                                              op0=ALU.mult)
                      psW = ps3s.tile([128, 8, 32], BF16, tag="pT", name="psW")[:, 0:4, :]
                      for j in range(4):
                          nc.tensor.transpose(psW[:, j, :],
                                              w_t[:, 128 * j:128 * (j + 1)],
                                              i128_t[0:32, 0:32])
                      wt_t = wk3.tile([128, 4, 32], BF16, tag="wt")
                      nc.vector.tensor_copy(wt_t[:], psW[:])

                      # ---- gate GEMM: psA=[i0,i1,f0,f1], psB=[o0,o1,g0,g1]
                      # x-MMs for this step were already emitted (pipelined,
                      # during the previous step's tail) into psA/psB.
                      psA, psB = ps_cur
                      units = [(0, 0), (0, 1), (1, 0), (1, 1),
                               (2, 0), (2, 1), (3, 0), (3, 1)]
                      nb = 4 + (1 if has_b else 0)
                      for k in range(8):
                          for u, (g, eta) in enumerate(units):
                              ps, j = (psA, u) if u < 4 else (psB, u - 4)
                              lo = 1024 * g + 512 * eta
                              sl = slice(32 * j, 32 * (j + 1))
                              nc.tensor.matmul(
                                  ps[sl, :], ht_lhs(t, k),
                                  wh_t[:, k, lo:lo + 512],
                                  start=False, stop=False,
                                  tile_position=(0, 32 * j),
                                  skip_group_check=True)
                      for c in range(4):
                          for u, (g, eta) in enumerate(units):
                              ps, j = (psA, u) if u < 4 else (psB, u - 4)
                              lo = 1024 * g + 512 * eta
                              sl = slice(32 * j, 32 * (j + 1))
                              nc.tensor.matmul(
                                  ps[sl, :], wt_t[:, c, :],
                                  bp_t[c][:, lo:lo + 512],
                                  start=False,
                                  stop=(c == nb - 1 and not has_b),
                                  tile_position=(0, 32 * j),
                                  skip_group_check=True)
                      if has_b:
                          for u, (g, eta) in enumerate(units):
                              ps, j = (psA, u) if u < 4 else (psB, u - 4)
                              lo = 1024 * g + 512 * eta
                              sl = slice(32 * j, 32 * (j + 1))
                              nc.tensor.matmul(
                                  ps[sl, :], ones_t[0:1, 0:32],
                                  bvec_t[0:1, lo:lo + 512],
                                  start=False, stop=True,
                                  tile_position=(0, 32 * j))

                      if t + 1 < S:
                          ps_cur = emit_x(t + 1)

                      # ---- gates
                      tif_t = wk3h.tile([128, 512], F32, tag="tif")
                      nc.scalar.activation(tif_t[:], psA[:], AF.Tanh, scale=0.5)
                      to_t = wk3h.tile([64, 512], F32, tag="to")
                      nc.scalar.activation(to_t[:], psB[0:64, :], AF.Tanh, scale=0.5)
                      tg_t = wk3h.tile([64, 512], F32, tag="tg")
                      nc.scalar.activation(tg_t[:], psB[64:128, :], AF.Tanh)
                      sf_t = wk3g.tile([64, 512], F32, tag="sf")
                      nc.vector.tensor_scalar(sf_t[:], tif_t[64:128, :], 0.5, 0.5,
                                              op0=ALU.mult, op1=ALU.add)
                      si_t = wk3g.tile([64, 512], F32, tag="si")
                      nc.vector.tensor_scalar(si_t[:], tif_t[0:64, :], 0.5, 0.5,
                                              op0=ALU.mult, op1=ALU.add)
                      so_t = wk3g.tile([64, 512], F32, tag="so")
                      nc.gpsimd.tensor_scalar(so_t[:], to_t[:], 0.5, 0.5,
                                              op0=ALU.mult, op1=ALU.add)
                      u_t = wk3g.tile([64, 512], F32, tag="u")
                      nc.vector.tensor_tensor(u_t[:], sf_t[:], c_t[:], op=ALU.mult)
                      v_t = wk3g.tile([64, 512], F32, tag="v")
                      nc.gpsimd.tensor_tensor(v_t[:], si_t[:], tg_t[:], op=ALU.mult)
                      nc.vector.tensor_tensor(c_t[:], u_t[:], v_t[:], op=ALU.add)
                      tc_t = wk3h.tile([64, 512], F32, tag="tc")
                      nc.scalar.activation(tc_t[:], c_t[:], AF.Tanh)
                      h0_t = wk3.tile([32, 512], BF16, tag="h0")
                      h1_t = wk3.tile([32, 512], BF16, tag="h1")
                      nc.vector.tensor_tensor(h0_t[:], so_t[0:32, :], tc_t[0:32, :],
                                              op=ALU.mult)
                      nc.gpsimd.tensor_tensor(h1_t[:], so_t[32:64, :], tc_t[32:64, :],
                                              op=ALU.mult)
                      psH = ps3s.tile([128, 8, 32], BF16, tag="pT")
                      for k in range(8):
                          src = h1_t if k >= 4 else h0_t
                          j = k % 4
                          nc.tensor.transpose(psH[:, k, :],
                                              src[:, 128 * j:128 * (j + 1)],
                                              i128_t[0:32, 0:32])
                      nc.vector.tensor_copy(
                          hst_t[:, :, B * t:B * (t + 1)], psH[:])
                      nc.scalar.copy(
                          hst8_t[:, :, B * t:B * (t + 1)], psH[:])

            # ================= P4: vocab scores -> loss ==========
            if phases == 2:
                dbg2_t = ppa.tile([1, 1], F32, tag="dbg2")
                nc.vector.tensor_copy(dbg2_t[:], c_t[0:1, 0:1])
                nc.sync.dma_start(d_loss[:], dbg2_t[:])
            if phases >= 3:
              with (
                  tc.tile_pool(name="p4", bufs=1) as p4,
                  tc.tile_pool(name="wk4", bufs=3) as wk4,
                  tc.tile_pool(name="ps4", bufs=4, space="PSUM") as ps4,
              ):
                  se_t = p4.tile([MROW, MCH, VCH], F32, tag="SE")
                  for vc in range(VCH):
                      wv_t = wk4.tile([128, 8, VCOL], FP8, tag="wv")
                      nc.sync.dma_start(
                          wv_t[:],
                          d_wvoc[:, VCOL * vc:VCOL * (vc + 1)]
                          .rearrange("(c k) m -> k c m", k=128))
                      if has_bvocab:
                          bvoc_t = wk4.tile([1, VCOL], BF16, tag="bvoc")
                          nc.sync.dma_start(
                              bvoc_t[:], d_bvoc[:, VCOL * vc:VCOL * (vc + 1)])
                      for m in range(MCH):
                          ps = ps4.tile([MROW, VCOL], F32, tag="pv")
                          for c in range(4):
                              nc.tensor.matmul(
                                  ps[:],
                                  hst8_t[:, 2 * c:2 * c + 2,
                                         MROW * m:MROW * (m + 1)],
                                  wv_t[:, 2 * c:2 * c + 2, :],
                                  start=(c == 0),
                                  stop=(c == 3 and not has_bvocab),
                                  perf_mode=DR)
                          if has_bvocab:
                              nc.tensor.matmul(
                                  ps[:], ones_t[0:1, 0:MROW], bvoc_t[0:1, :],
                                  start=False, stop=True)
                          scr = wk4.tile([MROW, VCOL], F32, tag="scr")
                          nc.scalar.activation(scr[:], ps[:], AF.Exp,
                                               accum_out=se_t[:, m, vc:vc + 1])

                  # target scores: sum over all rows of hsT*WtgtT (mask folded in)
                  wtgt_t = p4.tile([128, 8, ROWS], BF16, tag="wtgt")
                  nc.sync.dma_start(
                      wtgt_t[:], d_wtgt[:].rearrange("(c k) m -> k c m", k=128))
                  tparts = p4.tile([128, 8], F32, tag="tparts")
                  for k in range(8):
                      scr2 = wk4.tile([128, ROWS], F32, tag="scr2")
                      nc.vector.tensor_tensor(scr2[:], hst_t[:, k, :],
                                              wtgt_t[:, k, :], op=ALU.mult)
                      nc.vector.reduce_sum(tparts[:, k:k + 1], scr2[:],
                                           axis=AX.X)
                  tacc = p4.tile([128, 1], F32, tag="tacc")
                  nc.vector.reduce_sum(tacc[:], tparts[:], axis=AX.X)
                  tgt_r = p4.tile([128, 1], F32, tag="tgtr")
                  import concourse.bass_isa as bass_isa
                  nc.gpsimd.partition_all_reduce(tgt_r[:], tacc[:], channels=128,
                                                 reduce_op=bass_isa.ReduceOp.add)

                  # lse side
                  ses_t = p4.tile([MROW, MCH], F32, tag="ses")
                  nc.vector.reduce_sum(ses_t[:], se_t[:], axis=AX.X)
                  l_t = p4.tile([MROW, MCH], F32, tag="lt")
                  nc.scalar.activation(l_t[:], ses_t[:], AF.Ln)
                  maskm_t = p4.tile([MROW, MCH], F32, tag="maskm")
                  nc.sync.dma_start(maskm_t[:], d_maskm[:])
                  lm_t = p4.tile([MROW, MCH], F32, tag="lm")
                  nc.vector.tensor_tensor(lm_t[:], l_t[:], maskm_t[:], op=ALU.mult)
                  lr_t = p4.tile([MROW, 1], F32, tag="lr")
                  nc.vector.reduce_sum(lr_t[:], lm_t[:], axis=AX.X)
                  lse_r = p4.tile([MROW, 1], F32, tag="lser")
                  nc.gpsimd.partition_all_reduce(lse_r[:], lr_t[:], channels=MROW,
                                                 reduce_op=bass_isa.ReduceOp.add)

                  nll_t = p4.tile([1, 1], F32, tag="nll")
                  nc.vector.tensor_tensor(nll_t[:], lse_r[0:1, :], tgt_r[0:1, :],
                                          op=ALU.subtract)
                  if has_bvocab:
                      btgt_t = p4.tile([1, ROWS], F32, tag="btgt")
                      nc.sync.dma_start(btgt_t[:], d_btgt[:])
                      bts_t = p4.tile([1, 1], F32, tag="bts")
                      nc.vector.reduce_sum(bts_t[:], btgt_t[:], axis=AX.X)
                      nc.vector.tensor_tensor(nll_t[:], nll_t[:], bts_t[:],
                                              op=ALU.subtract)
                  loss_t = p4.tile([1, 1], F32, tag="loss")
                  nc.vector.tensor_scalar(loss_t[:], nll_t[:], 1.0 / N, None,
                                          op0=ALU.mult)
                  nc.sync.dma_start(d_loss[:], loss_t[:])

    nc.finalize()
    return nc


def kernel(features, captions, W_proj, b_proj, W_embed, Wx, Wh, Wattn, b,
           W_vocab, b_vocab):
    global last_exec_ns
    from concourse.bass_utils import run_bass_kernel_spmd

    features = np.asarray(features)
    captions = np.asarray(captions)
    cap_dtype = captions.dtype
    W_proj = np.asarray(W_proj, np.float32)
    b_proj = np.asarray(b_proj, np.float32)
    W_embed = np.asarray(W_embed, np.float32)
    Wx = np.asarray(Wx, np.float32)
    Wh = np.asarray(Wh, np.float32)
    Wattn = np.asarray(Wattn, np.float32)
    b = np.asarray(b, np.float32)
    W_vocab = np.asarray(W_vocab, np.float32)
    b_vocab = np.asarray(b_vocab, np.float32)

    has_b = bool(np.any(b))
    has_bvocab = bool(np.any(b_vocab))

    phases = int(os.environ.get("BASS_PHASES", "3"))
    key = (has_b, has_bvocab, phases)
    if key not in _cache:
        _cache[key] = _build(has_b, has_bvocab, phases)
    nc = _cache[key]

    cap_in = np.asarray(captions[:, :-1], np.int64)   # (N, S)
    cap_out = np.asarray(captions[:, 1:], np.int64)
    mask = (cap_out != 0).astype(np.float32)          # (N, S)
    x = W_embed[cap_in].astype(np.float32)            # (N, S, W_DIM)

    # shared (replicated) arrays
    wproj_h = np.zeros((1536, 1024), np.float32)
    wproj_h[:D_IMG] = W_proj
    wproj_h[D_IMG] = b_proj
    wproj_h = wproj_h.astype(F8)
    wh_h = Wh.astype(BF)
    wx_h = Wx.astype(BF)
    wattn_h = Wattn.astype(F8)
    wvoc_h = W_vocab.astype(F8)
    i128_h = np.eye(128, dtype=BF)
    col_n = np.arange(B * P16) // P16
    m32_h = np.where(col_n[None, :] == np.arange(B)[:, None], 0.0, NEG
                     ).astype(BF)
    bvec_h = b.reshape(1, 4096).astype(BF)
    bvoc_h = b_vocab.reshape(1, V).astype(BF)

    feat = features.reshape(N, D_IMG, P16).astype(np.float32)

    in_maps = []
    for ci in range(NC):
        sl = slice(ci * B, (ci + 1) * B)
        f2t = np.zeros((1536, 512), np.float32)
        f2t[:D_IMG] = feat[sl].transpose(1, 0, 2).reshape(D_IMG, B * P16)
        f2t[D_IMG] = 1.0
        xt = x[sl].transpose(2, 1, 0).reshape(W_DIM, ROWS)  # col = 32*t + n
        tgt = cap_out[sl].T.reshape(ROWS)                   # r = 32*t + n
        mk = mask[sl].T.reshape(ROWS)
        wtgt = (W_vocab[:, tgt] * mk[None, :]).astype(BF)
        maskm = mk.reshape(MCH, MROW).T.copy()              # [row, m]
        m = {
            "f2t": f2t.astype(F8),
            "wproj": wproj_h,
            "wattn": wattn_h,
            "wh": wh_h,
            "wx": wx_h,
            "xt": xt.astype(BF),
            "wvoc": wvoc_h,
            "wtgt": wtgt,
            "maskm": maskm.astype(np.float32),
            "i128": i128_h,
            "m32": m32_h,
        }
        if has_b:
            m["bvec"] = bvec_h
        if has_bvocab:
            m["bvoc"] = bvoc_h
            m["btgt"] = (b_vocab[tgt] * mk).reshape(1, ROWS).astype(np.float32)
        in_maps.append(m)

    trace = bool(int(os.environ.get("BASS_KPROF", "0")))
    if trace:
        import sys, types
        try:
            import antenv.axon_hooks  # noqa
        except ImportError:
            import trn_agent_boot.trn_boot as _tb
            _hook = _tb._ntff_profile_via_ctypes("/opt/axon/libaxon_pjrt.so")
            _mod = types.ModuleType("antenv.axon_hooks")
            _mod.get_axon_ntff_profile_hook = lambda: _hook
            import antenv
            sys.modules["antenv.axon_hooks"] = _mod
            antenv.axon_hooks = _mod

    if os.environ.get("BASS_SIM"):
        from concourse.bass_interp import CoreSim
        sim = CoreSim(nc)
        for k2, v2 in in_maps[0].items():
            sim.tensor(k2)[:] = v2
        sim.simulate()
        print("SIM core0 partial loss:", np.asarray(sim.tensor("loss"))[0, 0],
              flush=True)
        return np.asarray(np.float32(np.asarray(sim.tensor("loss"))[0, 0] * NC))

    res = run_bass_kernel_spmd(nc, in_maps, core_ids=list(range(NC)),
                               trace=trace)
    last_exec_ns = res.exec_time_ns
    total = np.float32(0.0)
    for ci in range(NC):
        total += res.results[ci]["loss"][0, 0]
    out = np.asarray(total, np.float32)
    del cap_dtype
    return out



# revision 33
# speedup vs baseline: 1.1998x; 1.1998x over previous
"""CaptioningRNN (attention LSTM + vocab softmax loss) on 8 TRN2 NeuronCores.

Data-parallel over batch N=256 -> 32 samples/core. Weights replicated.
All matmuls bf16 (fp32 PSUM accumulate). Per-core partial losses summed on host.

Layouts (per core, B=32 samples, S=31 steps, H=1024, P16=16 spatial):
  - GEMM orientation: out = lhsT.T @ rhs with lhsT = inputT slices
    (feature-dim on partitions, batch on lhsT free), rhs = weight slices
    (feature-dim on partitions, out-cols free). Gate psum tiles (128,512)
    pack 4 units of 32 batch rows via PE col-tiling (tile_position).
  - Attention uses an all-pairs score matmul masked by a constant M32
    (block-diag mask added via an identity matmul), softmax on ACT with
    fused accumulate, and a precomputed B = A2 @ Wattn so the per-step
    attention context enters the gate GEMM as 4 extra K-chunks.
"""

import os
import numpy as np
import ml_dtypes

BF = ml_dtypes.bfloat16
F8 = ml_dtypes.float8_e4m3  # IEEE e4m3 max±240 — matches TRN FP8_EXP4

N, T, V, W_DIM, H, D_IMG = 256, 32, 10000, 512, 1024, 1280
P16 = 16
NC = 8
B = N // NC          # 32 samples per core
S = T - 1            # 31 steps
ROWS = B * S         # 992 (t,n) rows per core, r = 32*t + n
MCH = 8              # vocab row chunks
MROW = ROWS // MCH   # 124
VCH = 20             # vocab col chunks
VCOL = V // VCH      # 500
NEG = -1.0e5         # mask value (exp underflows to exactly 0)

_cache = {}

last_exec_ns = None


def _build(has_b, has_bvocab, phases=3):
    import concourse.mybir as mybir
    from concourse.bacc import Bacc
    from concourse.tile import TileContext

    F32 = mybir.dt.float32
    BF16 = mybir.dt.bfloat16
    FP8 = mybir.dt.float8e4
    DR = mybir.MatmulPerfMode.DoubleRow
    AF = mybir.ActivationFunctionType
    ALU = mybir.AluOpType
    AX = mybir.AxisListType

    nc = Bacc()

    # ---- dram parameters (per-core shapes) ----
    d_f2t = nc.declare_dram_parameter("f2t", [1536, 512], FP8, isOutput=False)
    d_wproj = nc.declare_dram_parameter("wproj", [1536, 1024], FP8, isOutput=False)
    d_wattn = nc.declare_dram_parameter("wattn", [1024, 4096], FP8, isOutput=False)
    d_wh = nc.declare_dram_parameter("wh", [1024, 4096], BF16, isOutput=False)
    d_wx = nc.declare_dram_parameter("wx", [512, 4096], BF16, isOutput=False)
    d_xt = nc.declare_dram_parameter("xt", [512, ROWS], BF16, isOutput=False)
    d_wvoc = nc.declare_dram_parameter("wvoc", [1024, V], FP8, isOutput=False)
    d_wtgt = nc.declare_dram_parameter("wtgt", [1024, ROWS], BF16, isOutput=False)
    d_maskm = nc.declare_dram_parameter("maskm", [MROW, MCH], F32, isOutput=False)
    d_i128 = nc.declare_dram_parameter("i128", [128, 128], BF16, isOutput=False)
    d_m32 = nc.declare_dram_parameter("m32", [32, 512], BF16, isOutput=False)
    if has_b:
        d_bvec = nc.declare_dram_parameter("bvec", [1, 4096], BF16, isOutput=False)
    if has_bvocab:
        d_bvoc = nc.declare_dram_parameter("bvoc", [1, V], BF16, isOutput=False)
        d_btgt = nc.declare_dram_parameter("btgt", [1, ROWS], F32, isOutput=False)
    d_loss = nc.declare_dram_parameter("loss", [1, 1], F32, isOutput=True)

    with TileContext(nc) as tc:
        with tc.tile_pool(name="ppa", bufs=1) as ppa:
            # ---- persistent tiles (live across phases) ----
            at8_t = ppa.tile([128, 8, 512], FP8, tag="at8")       # A2T (H-chunks, (n,p))
            bp_t = [ppa.tile([128, 4096], BF16, tag=f"bp{c}", name=f"bp{c}") for c in range(4)]
            hst_t = ppa.tile([128, 8, ROWS], BF16, tag="hst")      # hsT history
            hst8_t = ppa.tile([128, 8, ROWS], FP8, tag="hst8")
            h0t_t = ppa.tile([128, 8, B], BF16, tag="h0t")
            h0t8_t = ppa.tile([128, 8, B], FP8, tag="h0t8")
            c_t = [ppa.tile([32, 512], F32, tag=f"cst{e}", name=f"cst{e}")
                   for e in range(2)]                              # LSTM c state
            i128_t = ppa.tile([128, 128], BF16, tag="i128")
            m32_t = ppa.tile([32, 512], BF16, tag="m32")
            ones_t = ppa.tile([1, 128], BF16, tag="ones")
            nc.sync.dma_start(i128_t[:], d_i128[:])
            nc.sync.dma_start(m32_t[:], d_m32[:])
            nc.vector.memset(ones_t[:], 1.0)
            if has_b:
                bvec_t = ppa.tile([1, 4096], BF16, tag="bvec")
                nc.sync.dma_start(bvec_t[:], d_bvec[:])

            # ================= P1: feature projection -> A2T, h0, c0 ==========
            with (
                tc.tile_pool(name="p12", bufs=1) as p12,
                tc.tile_pool(name="psa", bufs=2, space="PSUM") as psa,
            ):
                wproj_t = p12.tile([128, 12, 1024], FP8, tag="wproj")
                f2t_t = p12.tile([128, 12, 512], FP8, tag="f2t")
                h0f_t = p12.tile([128, 8, B], F32, tag="h0f")
                nc.sync.dma_start(
                    wproj_t[:], d_wproj[:].rearrange("(c k) m -> k c m", k=128))
                nc.sync.dma_start(
                    f2t_t[:], d_f2t[:].rearrange("(c k) m -> k c m", k=128))
                for hc in range(8):
                    ps = psa.tile([128, 512], F32, tag="pp")
                    for kk in range(6):
                        nc.tensor.matmul(
                            ps[:],
                            wproj_t[:, 2 * kk:2 * kk + 2,
                                    128 * hc:128 * (hc + 1)],
                            f2t_t[:, 2 * kk:2 * kk + 2, :],
                            start=(kk == 0), stop=(kk == 5), perf_mode=DR)
                    nc.scalar.copy(at8_t[:, hc, :], ps[:])
                    nc.vector.reduce_sum(
                        h0f_t[:, hc, :],
                        ps[:].rearrange("k (n p) -> k n p", p=P16),
                        axis=AX.X)
                nc.vector.tensor_scalar(h0t_t[:], h0f_t[:],
                                        1.0 / P16, None, op0=ALU.mult)
                nc.gpsimd.tensor_scalar(h0t8_t[:], h0f_t[:],
                                        1.0 / P16, None, op0=ALU.mult)
                for eta in range(2):
                    c0p = psa.tile([32, 512], BF16, tag="c0p")
                    for j in range(4):
                        nc.tensor.transpose(
                            c0p[:, 128 * j:128 * (j + 1)],
                            h0t_t[:, 4 * eta + j, :], i128_t[:, 0:128])
                    nc.vector.tensor_copy(c_t[eta][:], c0p[:])

                # ================= P2: B = A2 @ Wattn ==========
                with tc.tile_pool(name="p2w", bufs=2) as p2w:
                    for nch in range(8):
                        wat_t = p2w.tile([128, 8, 512], FP8, tag="wat")
                        nc.sync.dma_start(
                            wat_t[:],
                            d_wattn[:, 512 * nch:512 * (nch + 1)]
                            .rearrange("(c k) m -> k c m", k=128))
                        for c in range(4):
                            ps = psa.tile([128, 512], F32, tag="pp")
                            for k in range(4):
                                nc.tensor.matmul(
                                    ps[:],
                                    at8_t[:, 2 * k:2 * k + 2,
                                          128 * c:128 * (c + 1)],
                                    wat_t[:, 2 * k:2 * k + 2, :],
                                    start=(k == 0), stop=(k == 3),
                                    perf_mode=DR)
                            nc.vector.tensor_copy(
                                bp_t[c][:, 512 * nch:512 * (nch + 1)], ps[:])

            # ================= P3: recurrence ==========
            if phases < 2:
                dbg_t = ppa.tile([1, 1], F32, tag="dbg")
                nc.vector.tensor_copy(dbg_t[:], c_t[0][0:1, 0:1])
                nc.sync.dma_start(d_loss[:], dbg_t[:])
            if phases >= 2:
              with (
                  tc.tile_pool(name="ppb", bufs=1) as ppb,
                  tc.tile_pool(name="ps3", bufs=2, space="PSUM") as ps3,
                  tc.tile_pool(name="ps3s", bufs=2, space="PSUM") as ps3s,
                  tc.tile_pool(name="wk3", bufs=3) as wk3,
                  tc.tile_pool(name="wk3g", bufs=1) as wk3g,
                  tc.tile_pool(name="wk3h", bufs=1) as wk3h,
              ):
                  wh_t = ppb.tile([128, 8, 4096], BF16, tag="wh")
                  wx_t = ppb.tile([128, 4, 4096], BF16, tag="wx")
                  xt_t = ppb.tile([128, 4, ROWS], BF16, tag="xt")
                  nc.sync.dma_start(
                      wh_t[:], d_wh[:].rearrange("(c k) m -> k c m", k=128))
                  nc.sync.dma_start(
                      wx_t[:], d_wx[:].rearrange("(c k) m -> k c m", k=128))
                  nc.sync.dma_start(
                      xt_t[:], d_xt[:].rearrange("(c k) m -> k c m", k=128))

                  def ht_lhs(t, k):
                      if t == 0:
                          return h0t_t[:, k, :]
                      return hst_t[:, k, B * (t - 1):B * t]

                  def ht8_pair(t, c):
                      if t == 0:
                          return h0t8_t[:, 2 * c:2 * c + 2, :]
                      return hst8_t[:, 2 * c:2 * c + 2, B * (t - 1):B * t]

                  # unit layout: ps[eta] rows = [i, f, o, g] (32 each), col grp
                  # j = gate index, gate cols lo = 1024*gi + 512*eta
                  def emit_x(t2):
                      pp = [ps3.tile([128, 512], F32, tag=f"p{e}",
                                     name=f"p{e}_{t2}") for e in range(2)]
                      for c2 in range(4):
                          for e2 in range(2):
                              for g2 in range(4):
                                  lo2 = 1024 * g2 + 512 * e2
                                  nc.tensor.matmul(
                                      pp[e2][32 * g2:32 * (g2 + 1), :],
                                      xt_t[:, c2, B * t2:B * (t2 + 1)],
                                      wx_t[:, c2, lo2:lo2 + 512],
                                      start=(c2 == 0), stop=False,
                                      tile_position=(0, 32 * g2),
                                      skip_group_check=True)
                      return pp

                  ps_cur = emit_x(0)
                  for t in range(S):
                      # ---- attention scores + softmax (uses h from step t-1)
                      psS = ps3s.tile([32, 512], F32, tag="pS")
                      nc.tensor.matmul(psS[:], i128_t[0:32, 0:32], m32_t[:],
                                       start=True, stop=False)
                      for c in range(4):
                          nc.tensor.matmul(psS[:], ht8_pair(t, c),
                                           at8_t[:, 2 * c:2 * c + 2, :],
                                           start=False, stop=(c == 3),
                                           perf_mode=DR)
                      e_t = wk3.tile([32, 512], F32, tag="e")
                      se_t = wk3.tile([32, 1], F32, tag="se")
                      nc.scalar.activation(e_t[:], psS[:], AF.Exp,
                                           scale=float(1.0 / np.sqrt(H)),
                                           accum_out=se_t[:, 0:1])
                      re_t = wk3.tile([32, 1], F32, tag="re")
                      nc.vector.reciprocal(re_t[:], se_t[:])
                      w_t = wk3.tile([32, 512], BF16, tag="w")
                      nc.vector.tensor_scalar(w_t[:], e_t[:], re_t[:, 0:1], None,
                                              op0=ALU.mult)

                      # ---- gate GEMM. PE order: Wh(e0) | wT transposes |
                      # attn(e0) | Wh(e1) | attn(e1) | x(t+1) | hT(e0) | hT(e1)
                      # so softmax/eltwise (ACT/DVE) overlap the MM stream.
                      def emit_wh(eta):
                          ps = ps_cur[eta]
                          for k in range(8):
                              for g in range(4):
                                  lo = 1024 * g + 512 * eta
                                  nc.tensor.matmul(
                                      ps[32 * g:32 * (g + 1), :],
                                      ht_lhs(t, k), wh_t[:, k, lo:lo + 512],
                                      start=False, stop=False,
                                      tile_position=(0, 32 * g),
                                      skip_group_check=True)

                      def emit_attn(eta):
                          ps = ps_cur[eta]
                          for c in range(4):
                              for g in range(4):
                                  lo = 1024 * g + 512 * eta
                                  nc.tensor.matmul(
                                      ps[32 * g:32 * (g + 1), :],
                                      wt_t[:, c, :], bp_t[c][:, lo:lo + 512],
                                      start=False,
                                      stop=(c == 3 and not has_b),
                                      tile_position=(0, 32 * g),
                                      skip_group_check=True)
                          if has_b:
                              for g in range(4):
                                  lo = 1024 * g + 512 * eta
                                  nc.tensor.matmul(
                                      ps[32 * g:32 * (g + 1), :],
                                      ones_t[0:1, 0:32],
                                      bvec_t[0:1, lo:lo + 512],
                                      start=False, stop=True,
                                      tile_position=(0, 32 * g))

                      def emit_elt(eta, psH):
                          ps = ps_cur[eta]
                          sf_ = wk3g.tile([32, 512], F32, tag=f"sf{eta}",
                                          name=f"sf{eta}")
                          nc.scalar.activation(sf_[:], ps[32:64, :], AF.Sigmoid)
                          si_ = wk3g.tile([32, 512], F32, tag=f"si{eta}",
                                          name=f"si{eta}")
                          nc.scalar.activation(si_[:], ps[0:32, :], AF.Sigmoid)
                          tg_ = wk3h.tile([32, 512], F32, tag=f"tg{eta}",
                                          name=f"tg{eta}")
                          nc.scalar.activation(tg_[:], ps[96:128, :], AF.Tanh)
                          so_ = wk3g.tile([32, 512], F32, tag=f"so{eta}",
                                          name=f"so{eta}")
                          nc.scalar.activation(so_[:], ps[64:96, :], AF.Sigmoid)
                          nc.vector.tensor_tensor(sf_[:], sf_[:],
                                                  c_t[eta][:], op=ALU.mult)
                          nc.gpsimd.tensor_tensor(si_[:], si_[:], tg_[:],
                                                  op=ALU.mult)
                          nc.vector.tensor_tensor(c_t[eta][:], sf_[:], si_[:],
                                                  op=ALU.add)
                          tc_ = wk3h.tile([32, 512], F32, tag=f"tc{eta}",
                                          name=f"tc{eta}")
                          nc.scalar.activation(tc_[:], c_t[eta][:], AF.Tanh)
                          h_ = wk3.tile([32, 512], BF16, tag=f"h{eta}",
                                        name=f"h{eta}")
                          nc.vector.tensor_tensor(h_[:], so_[:], tc_[:],
                                                  op=ALU.mult)
                          return h_

                      def emit_ht(eta, h_, psH):
                          for j in range(4):
                              nc.tensor.transpose(
                                  psH[:, 4 * eta + j, :],
                                  h_[:, 128 * j:128 * (j + 1)],
                                  i128_t[0:32, 0:32])
                          nc.scalar.copy(
                              hst8_t[:, 4 * eta:4 * eta + 4, B * t:B * (t + 1)],
                              psH[:, 4 * eta:4 * eta + 4, :])
                          nc.vector.tensor_copy(
                              hst_t[:, 4 * eta:4 * eta + 4, B * t:B * (t + 1)],
                              psH[:, 4 * eta:4 * eta + 4, :])

                      psH = ps3s.tile([128, 8, 32], BF16, tag="pT")
                      emit_wh(0)
                      psW = ps3s.tile([128, 8, 32], BF16, tag="pT",
                                      name="psW")[:, 0:4, :]
                      for j in range(4):
                          nc.tensor.transpose(psW[:, j, :],
                                              w_t[:, 128 * j:128 * (j + 1)],
                                              i128_t[0:32, 0:32])
                      wt_t = wk3.tile([128, 4, 32], BF16, tag="wt")
                      nc.vector.tensor_copy(wt_t[:], psW[:])
                      emit_attn(0)
                      h0_ = emit_elt(0, psH)
                      emit_wh(1)
                      emit_attn(1)
                      h1_ = emit_elt(1, psH)
                      if t + 1 < S:
                          ps_nxt = emit_x(t + 1)
                      emit_ht(0, h0_, psH)
                      emit_ht(1, h1_, psH)
                      if t + 1 < S:
                          ps_cur = ps_nxt

            # ================= P4: vocab scores -> loss ==========
            if phases == 2:
                dbg2_t = ppa.tile([1, 1], F32, tag="dbg2")
                nc.vector.tensor_copy(dbg2_t[:], c_t[0][0:1, 0:1])
                nc.sync.dma_start(d_loss[:], dbg2_t[:])
            if phases >= 3:
              with (
                  tc.tile_pool(name="p4", bufs=1) as p4,
                  tc.tile_pool(name="wk4", bufs=3) as wk4,
                  tc.tile_pool(name="ps4", bufs=4, space="PSUM") as ps4,
              ):
                  se_t = p4.tile([MROW, MCH, VCH], F32, tag="SE")
                  for vc in range(VCH):
                      wv_t = wk4.tile([128, 8, VCOL], FP8, tag="wv")
                      nc.sync.dma_start(
                          wv_t[:],
                          d_wvoc[:, VCOL * vc:VCOL * (vc + 1)]
                          .rearrange("(c k) m -> k c m", k=128))
                      if has_bvocab:
                          bvoc_t = wk4.tile([1, VCOL], BF16, tag="bvoc")
                          nc.sync.dma_start(
                              bvoc_t[:], d_bvoc[:, VCOL * vc:VCOL * (vc + 1)])
                      for m in range(MCH):
                          ps = ps4.tile([MROW, VCOL], F32, tag="pv")
                          for c in range(4):
                              nc.tensor.matmul(
                                  ps[:],
                                  hst8_t[:, 2 * c:2 * c + 2,
                                         MROW * m:MROW * (m + 1)],
                                  wv_t[:, 2 * c:2 * c + 2, :],
                                  start=(c == 0),
                                  stop=(c == 3 and not has_bvocab),
                                  perf_mode=DR)
                          if has_bvocab:
                              nc.tensor.matmul(
                                  ps[:], ones_t[0:1, 0:MROW], bvoc_t[0:1, :],
                                  start=False, stop=True)
                          scr = wk4.tile([MROW, VCOL], F32, tag="scr")
                          nc.scalar.activation(scr[:], ps[:], AF.Exp,
                                               accum_out=se_t[:, m, vc:vc + 1])

                  # target scores: sum over all rows of hsT*WtgtT (mask folded in)
                  wtgt_t = p4.tile([128, 8, ROWS], BF16, tag="wtgt")
                  nc.sync.dma_start(
                      wtgt_t[:], d_wtgt[:].rearrange("(c k) m -> k c m", k=128))
                  tparts = p4.tile([128, 8], F32, tag="tparts")
                  for k in range(8):
                      scr2 = wk4.tile([128, ROWS], F32, tag="scr2")
                      nc.vector.tensor_tensor(scr2[:], hst_t[:, k, :],
                                              wtgt_t[:, k, :], op=ALU.mult)
                      nc.vector.reduce_sum(tparts[:, k:k + 1], scr2[:],
                                           axis=AX.X)
                  tacc = p4.tile([128, 1], F32, tag="tacc")
                  nc.vector.reduce_sum(tacc[:], tparts[:], axis=AX.X)
                  tgt_r = p4.tile([128, 1], F32, tag="tgtr")
                  import concourse.bass_isa as bass_isa
                  nc.gpsimd.partition_all_reduce(tgt_r[:], tacc[:], channels=128,
                                                 reduce_op=bass_isa.ReduceOp.add)

                  # lse side
                  ses_t = p4.tile([MROW, MCH], F32, tag="ses")
                  nc.vector.reduce_sum(ses_t[:], se_t[:], axis=AX.X)
                  l_t = p4.tile([MROW, MCH], F32, tag="lt")
                  nc.scalar.activation(l_t[:], ses_t[:], AF.Ln)
                  maskm_t = p4.tile([MROW, MCH], F32, tag="maskm")
                  nc.sync.dma_start(maskm_t[:], d_maskm[:])
                  lm_t = p4.tile([MROW, MCH], F32, tag="lm")
                  nc.vector.tensor_tensor(lm_t[:], l_t[:], maskm_t[:], op=ALU.mult)
                  lr_t = p4.tile([MROW, 1], F32, tag="lr")
                  nc.vector.reduce_sum(lr_t[:], lm_t[:], axis=AX.X)
                  lse_r = p4.tile([MROW, 1], F32, tag="lser")
                  nc.gpsimd.partition_all_reduce(lse_r[:], lr_t[:], channels=MROW,
                                                 reduce_op=bass_isa.ReduceOp.add)

                  nll_t = p4.tile([1, 1], F32, tag="nll")
                  nc.vector.tensor_tensor(nll_t[:], lse_r[0:1, :], tgt_r[0:1, :],
                                          op=ALU.subtract)
                  if has_bvocab:
                      btgt_t = p4.tile([1, ROWS], F32, tag="btgt")
                      nc.sync.dma_start(btgt_t[:], d_btgt[:])
                      bts_t = p4.tile([1, 1], F32, tag="bts")
                      nc.vector.reduce_sum(bts_t[:], btgt_t[:], axis=AX.X)
                      nc.vector.tensor_tensor(nll_t[:], nll_t[:], bts_t[:],
                                              op=ALU.subtract)
                  loss_t = p4.tile([1, 1], F32, tag="loss")
                  nc.vector.tensor_scalar(loss_t[:], nll_t[:], 1.0 / N, None,
                                          op0=ALU.mult)
                  nc.sync.dma_start(d_loss[:], loss_t[:])

    nc.finalize()
    return nc


def kernel(features, captions, W_proj, b_proj, W_embed, Wx, Wh, Wattn, b,
           W_vocab, b_vocab):
    global last_exec_ns
    from concourse.bass_utils import run_bass_kernel_spmd

    features = np.asarray(features)
    captions = np.asarray(captions)
    cap_dtype = captions.dtype
    W_proj = np.asarray(W_proj, np.float32)
    b_proj = np.asarray(b_proj, np.float32)
    W_embed = np.asarray(W_embed, np.float32)
    Wx = np.asarray(Wx, np.float32)
    Wh = np.asarray(Wh, np.float32)
    Wattn = np.asarray(Wattn, np.float32)
    b = np.asarray(b, np.float32)
    W_vocab = np.asarray(W_vocab, np.float32)
    b_vocab = np.asarray(b_vocab, np.float32)

    has_b = bool(np.any(b))
    has_bvocab = bool(np.any(b_vocab))

    phases = int(os.environ.get("BASS_PHASES", "3"))
    key = (has_b, has_bvocab, phases)
    if key not in _cache:
        _cache[key] = _build(has_b, has_bvocab, phases)
    nc = _cache[key]

    cap_in = np.asarray(captions[:, :-1], np.int64)   # (N, S)
    cap_out = np.asarray(captions[:, 1:], np.int64)
    mask = (cap_out != 0).astype(np.float32)          # (N, S)
    x = W_embed[cap_in].astype(np.float32)            # (N, S, W_DIM)

    # shared (replicated) arrays
    wproj_h = np.zeros((1536, 1024), np.float32)
    wproj_h[:D_IMG] = W_proj
    wproj_h[D_IMG] = b_proj
    wproj_h = wproj_h.astype(F8)
    wh_h = Wh.astype(BF)
    wx_h = Wx.astype(BF)
    wattn_h = Wattn.astype(F8)
    wvoc_h = W_vocab.astype(F8)
    i128_h = np.eye(128, dtype=BF)
    col_n = np.arange(B * P16) // P16
    m32_h = np.where(col_n[None, :] == np.arange(B)[:, None], 0.0, NEG
                     ).astype(BF)
    bvec_h = b.reshape(1, 4096).astype(BF)
    bvoc_h = b_vocab.reshape(1, V).astype(BF)

    feat = features.reshape(N, D_IMG, P16).astype(np.float32)

    in_maps = []
    for ci in range(NC):
        sl = slice(ci * B, (ci + 1) * B)
        f2t = np.zeros((1536, 512), np.float32)
        f2t[:D_IMG] = feat[sl].transpose(1, 0, 2).reshape(D_IMG, B * P16)
        f2t[D_IMG] = 1.0
        xt = x[sl].transpose(2, 1, 0).reshape(W_DIM, ROWS)  # col = 32*t + n
        tgt = cap_out[sl].T.reshape(ROWS)                   # r = 32*t + n
        mk = mask[sl].T.reshape(ROWS)
        wtgt = (W_vocab[:, tgt] * mk[None, :]).astype(BF)
        maskm = mk.reshape(MCH, MROW).T.copy()              # [row, m]
        m = {
            "f2t": f2t.astype(F8),
            "wproj": wproj_h,
            "wattn": wattn_h,
            "wh": wh_h,
            "wx": wx_h,
            "xt": xt.astype(BF),
            "wvoc": wvoc_h,
            "wtgt": wtgt,
            "maskm": maskm.astype(np.float32),
            "i128": i128_h,
            "m32": m32_h,
        }
        if has_b:
            m["bvec"] = bvec_h
        if has_bvocab:
            m["bvoc"] = bvoc_h
            m["btgt"] = (b_vocab[tgt] * mk).reshape(1, ROWS).astype(np.float32)
        in_maps.append(m)

    trace = bool(int(os.environ.get("BASS_KPROF", "0")))
    if trace:
        import sys, types
        try:
            import antenv.axon_hooks  # noqa
        except ImportError:
            import trn_agent_boot.trn_boot as _tb
            _hook = _tb._ntff_profile_via_ctypes("/opt/axon/libaxon_pjrt.so")
            _mod = types.ModuleType("antenv.axon_hooks")
            _mod.get_axon_ntff_profile_hook = lambda: _hook
            import antenv
            sys.modules["antenv.axon_hooks"] = _mod
            antenv.axon_hooks = _mod

    if os.environ.get("BASS_SIM"):
        from concourse.bass_interp import CoreSim
        sim = CoreSim(nc)
        for k2, v2 in in_maps[0].items():
            sim.tensor(k2)[:] = v2
        sim.simulate()
        print("SIM core0 partial loss:", np.asarray(sim.tensor("loss"))[0, 0],
              flush=True)
        return np.asarray(np.float32(np.asarray(sim.tensor("loss"))[0, 0] * NC))

    res = run_bass_kernel_spmd(nc, in_maps, core_ids=list(range(NC)),
                               trace=trace)
    last_exec_ns = res.exec_time_ns
    total = np.float32(0.0)
    for ci in range(NC):
        total += res.results[ci]["loss"][0, 0]
    out = np.asarray(total, np.float32)
    del cap_dtype
    return out

